# revision 39
# baseline (speedup 1.0000x reference)
"""Attention kernel for trn2: B=4, N=2048, DIM=512, HEADS=8, DIM_HEAD=64.

Sharding: head-parallel across 8 cores (core h computes head h for all 4
batches). Each core returns its head's contribution through W_out as a
bf16 partial [4, 2048, 512]; the host sums the 8 partials in fp32.

Per-core pipeline (bf16 data path, fp32 PSUM accumulation):
  phase 1: qkT = (W_qk)^T x^T computed directly in [d, n] layout; rotary
           applied via a +-1 permutation matmul (pair swap) plus 3 DVE
           ops against precomputed cos/sin tables; v in natural [n, d]
           layout with a ones column for row sums.
  phase 2: every matmul is contract-64 and runs 2x via PE row tiling
           (two concurrent 64x128 sub-array tiles at partitions 0:64 /
           64:128):
             S^T[k,q]: even key-chunks on tile (0,0) (k from kTlow,
             q from qkT rows 0:64), odd chunks on tile (64,0) (k from
             qkT rows 64:128, q from a duplicated copy at rows 64:128).
             exp on ACT (1024 wide), attn = et * exp(bias) on DVE (bf16,
             exp(bias) precomputed on host).
             PV: each 128-key chunk split into two 64-key halves run
             concurrently on both tiles into separate accumulators
             outT_A/outT_B (summed in the epilogue).
             W_out: two 128-row chunks run concurrently (ho duplicated
             to partitions 64:128 by SBUF-to-SBUF DMA).
"""

import itertools

import numpy as np

B, N, DIM = 4, 2048, 512
HEADS, DH = 8, 64
P = 128
DC = DIM // P          # 4 input-dim chunks
KC = N // P            # 16 key chunks
QT = 512               # query tile in phase 2
NQT = N // QT          # 4
NPAIR = KC // 2        # 8 key-chunk pairs

_CACHE = {}


def _build():
    import concourse.mybir as mybir
    import concourse.tile as tile
    from concourse import bacc

    F32 = mybir.dt.float32
    F32R = mybir.dt.float32r
    BF16 = mybir.dt.bfloat16

    nc = bacc.Bacc(None, target_bir_lowering=False)

    xT4_d = nc.dram_tensor("xT4", [B, P, DC, N], BF16, kind="ExternalInput")
    wqk_d = nc.dram_tensor("wqk", [P, DC, P], BF16, kind="ExternalInput")
    wv_d = nc.dram_tensor("wv", [P, DC, DH], BF16, kind="ExternalInput")
    st_d = nc.dram_tensor("st", [P, P], BF16, kind="ExternalInput")
    cosT_d = nc.dram_tensor("cosT", [P, N], BF16, kind="ExternalInput")
    sinT_d = nc.dram_tensor("sinT", [P, N], BF16, kind="ExternalInput")
    expb_d = nc.dram_tensor("expb", [KC, P, N], BF16, kind="ExternalInput")
    woutw_d = nc.dram_tensor("woutw", [DH, DIM], BF16, kind="ExternalInput")
    ones64_d = nc.dram_tensor("ones64", [DH, P], F32R, kind="ExternalInput")
    zeros64_d = nc.dram_tensor("zeros64", [DH, QT], F32R, kind="ExternalInput")
    out_d = nc.dram_tensor("out", [B, N, DIM], BF16, kind="ExternalOutput")

    with tile.TileContext(nc) as tc:
        with tc.tile_pool(name="const", bufs=1) as cp:
            wqk_t = cp.tile([P, DC, P], BF16, tag="wqk")
            nc.sync.dma_start(wqk_t[:], wqk_d[:, :, :])
            wv_t = cp.tile([P, DC, DH], BF16, tag="wv")
            nc.sync.dma_start(wv_t[:], wv_d[:, :, :])
            st_t = cp.tile([P, P], BF16, tag="st")
            nc.sync.dma_start(st_t[:], st_d[:, :])
            cosT_t = cp.tile([P, N], BF16, tag="cosT")
            sinT_t = cp.tile([P, N], BF16, tag="sinT")
            nc.sync.dma_start(cosT_t[:], cosT_d[:, :])
            nc.sync.dma_start(sinT_t[:], sinT_d[:, :])
            # W_out rhs duplicated into both partition halves for the
            # row-tiled wout pair
            woutw2_t = cp.tile([P, DIM], BF16, tag="woutw2")
            nc.sync.dma_start(woutw2_t[0:DH, :], woutw_d[:, :])
            nc.sync.dma_start(woutw2_t[DH:P, :], woutw_d[:, :])
            ones64_t = cp.tile([DH, P], F32R, tag="ones64")
            nc.sync.dma_start(ones64_t[:], ones64_d[:, :])
            # row-sum staging: row 0 written per (b, jq); rows 1:64 stay 0
            rs64_t = [
                cp.tile([DH, QT], F32R, tag=f"rs64_{i}", name=f"rs64_{i}")
                for i in range(2)
            ]
            for t in rs64_t:
                nc.sync.dma_start(t[:], zeros64_d[:, :])
            expb_t = [
                cp.tile([P, 2, N], BF16, tag=f"expb{t}", name=f"expb{t}")
                for t in range(NPAIR)
            ]

            def load_expb(ts):
                # staggered through phase 1 so the 8.4MB doesn't contend
                # with the startup-critical x/weight loads
                for t in ts:
                    nc.scalar.dma_start(
                        expb_t[t][:],
                        expb_d[2 * t : 2 * t + 2, :, :].rearrange("k p n -> p k n"),
                    )

            qkT_b = [
                cp.tile([P, N], BF16, tag=f"qkT{b}", name=f"qkT{b}") for b in range(B)
            ]
            kTlow_b = [
                cp.tile([DH, N], BF16, tag=f"kTlow{b}", name=f"kTlow{b}")
                for b in range(B)
            ]
            qhi_b = [
                cp.tile([P, N], BF16, tag=f"qhi{b}", name=f"qhi{b}")
                for b in range(B)
            ]
            v_b = [
                cp.tile([P, KC, DH + 2], BF16, tag=f"v{b}", name=f"v{b}")
                for b in range(B)
            ]

            # ---- phase 1: qkv projection (transposed) + rotary ----
            with (
                tc.tile_pool(name="p1", bufs=2) as p1,
                tc.tile_pool(name="pp1", bufs=2, space="PSUM") as pp1,
            ):
                for b in range(B):
                    xt = p1.tile([P, DC, N], BF16, tag="xt")
                    for mc in range(N // QT):
                        ms = slice(mc * QT, (mc + 1) * QT)
                        nc.sync.dma_start(xt[:, :, ms], xT4_d[b, :, :, ms])
                    for mc in range(N // QT):
                        ms = slice(mc * QT, (mc + 1) * QT)
                        qk_ps = pp1.tile([P, QT], F32, tag="qkps")
                        for dc in range(DC):
                            nc.tensor.matmul(
                                qk_ps[:],
                                lhsT=wqk_t[:, dc, :],
                                rhs=xt[:, dc, ms],
                                start=(dc == 0),
                                stop=(dc == DC - 1),
                            )
                        qk_sb = p1.tile([P, QT], BF16, tag="qksb")
                        nc.scalar.copy(qk_sb[:], qk_ps[:])
                        rot_ps = pp1.tile([P, QT], F32, tag="rotps")
                        nc.tensor.matmul(
                            rot_ps[:], lhsT=st_t[:], rhs=qk_sb[:],
                            start=True, stop=True,
                        )
                        # v for the four 128-row chunks of this column block;
                        # interleaved here so their weight loads hide under
                        # the long qk/rot matmuls
                        v_ps = pp1.tile([P, 4, DH], F32, tag="vps")
                        for i in range(4):
                            ch = mc * 4 + i
                            for dc in range(DC):
                                nc.tensor.matmul(
                                    v_ps[:, i, :],
                                    lhsT=xt[:, dc, ch * P : (ch + 1) * P],
                                    rhs=wv_t[:, dc, :],
                                    start=(dc == 0),
                                    stop=(dc == DC - 1),
                                )
                        a_t = p1.tile([P, QT], BF16, tag="a")
                        nc.vector.tensor_mul(a_t[:], qk_sb[:], cosT_t[:, ms])
                        rb_t = p1.tile([P, QT], BF16, tag="rb")
                        nc.vector.tensor_mul(rb_t[:], rot_ps[:], sinT_t[:, ms])
                        nc.vector.tensor_add(qkT_b[b][:, ms], a_t[:], rb_t[:])
                        # matmul operands must share a base partition: stage
                        # k at partitions 0:64 and a q copy at 64:128
                        nc.sync.dma_start(kTlow_b[b][:, ms], qkT_b[b][DH:P, ms])
                        nc.sync.dma_start(
                            qhi_b[b][DH:P, ms], qkT_b[b][0:DH, ms]
                        )
                        nc.scalar.copy(
                            v_b[b][:, mc * 4 : (mc + 1) * 4, 0:DH], v_ps[:]
                        )
                    nc.vector.memset(v_b[b][:, :, DH : DH + 1], 1.0)
                    load_expb(range(2 * b, 2 * b + 2))

            # ---- phase 2: attention (all matmuls 64x128 row-tiled) ----
            with (
                tc.tile_pool(name="p2", bufs=3) as p2,
                tc.tile_pool(name="ps2", bufs=2, space="PSUM") as ps2,
                tc.tile_pool(name="psa", bufs=1, space="PSUM") as psa,
                tc.tile_pool(name="psb", bufs=1, space="PSUM") as psb,
                tc.tile_pool(name="psw", bufs=2, space="PSUM") as psw,
            ):

                def epilogue_tail(b, jq, rs, outA, outB):
                    """Epilogue steps, one dependency hop per chunk so no
                    engine queue head-of-line blocks ahead of the next
                    block's exp/mult work. Both accumulators are drained
                    concurrently (ACT || DVE) so the next block's first PV
                    pair is released as fast as possible."""
                    outA_sb = p2.tile(
                        [DH + 1, QT], F32, tag="outAsb", name="outA_sb", bufs=2
                    )
                    nc.scalar.copy(outA_sb[:], outA[:])
                    outB_sb = p2.tile(
                        [DH + 1, QT], F32, tag="outBsb", name="outB_sb", bufs=2
                    )
                    nc.vector.tensor_copy(outB_sb[:], outB[:])
                    yield
                    with nc.allow_low_precision(reason="f32r row-sum staging"):
                        nc.vector.tensor_add(
                            rs[0:1, :],
                            outA_sb[DH : DH + 1, :],
                            outB_sb[DH : DH + 1, :],
                        )
                    hsum = p2.tile([DH, QT], F32, tag="hsum", name="hsum", bufs=2)
                    nc.vector.tensor_add(
                        hsum[:], outA_sb[0:DH, :], outB_sb[0:DH, :]
                    )
                    yield
                    bc_ps = psw.tile([P, QT], F32, tag="wo", name="bc_ps")
                    nc.tensor.matmul(
                        bc_ps[:], lhsT=ones64_t[:], rhs=rs[:],
                        start=True, stop=True,
                    )
                    yield
                    bcr_sb = p2.tile([DH, QT], F32, tag="bcr", name="bcr_sb", bufs=2)
                    nc.vector.reciprocal(bcr_sb[:], bc_ps[0:DH, :])
                    yield
                    ho_t = p2.tile([P, QT], BF16, tag="ho", name="ho_t", bufs=2)
                    nc.vector.tensor_mul(ho_t[0:DH, :], hsum[:], bcr_sb[:])
                    yield
                    nc.sync.dma_start(ho_t[DH:P, :], ho_t[0:DH, :])
                    yield
                    for sqp in range(2):
                        wo_lo = psw.tile([P, DIM], F32, tag="wo", name="wo_lo")
                        nc.tensor.matmul(
                            wo_lo[:],
                            lhsT=ho_t[0:DH, (2 * sqp) * P : (2 * sqp + 1) * P],
                            rhs=woutw2_t[0:DH, :],
                            start=True,
                            stop=True,
                        )
                        wo_hi = psw.tile([P, DIM], F32, tag="wo", name="wo_hi")
                        nc.tensor.matmul(
                            wo_hi[:],
                            lhsT=ho_t[DH:P, (2 * sqp + 1) * P : (2 * sqp + 2) * P],
                            rhs=woutw2_t[DH:P, :],
                            start=True,
                            stop=True,
                        )
                        yield
                        for half, wo_ps in enumerate((wo_lo, wo_hi)):
                            ob = p2.tile([P, DIM], BF16, tag="ob", name="ob")
                            nc.scalar.copy(ob[:], wo_ps[:])
                            row0 = jq * QT + (2 * sqp + half) * P
                            nc.sync.dma_start(out_d[b, row0 : row0 + P, :], ob[:])
                        yield

                # one continuous software-pipelined stream over all
                # (b, jq, pair) steps: PV lags S/exp/mult by LAG pairs even
                # across (b, jq) boundaries, so the S->exp->mult->PV chain
                # latency (~3us) amortizes instead of stalling each pair
                LAG = 4
                pending = None
                inflight = []
                outA = outB = None
                total = B * NQT * NPAIR
                for gi in range(total + LAG):
                    if gi < total:
                        bji, pp = divmod(gi, NPAIR)
                        b, jq = divmod(bji, NQT)
                        qs = slice(jq * QT, (jq + 1) * QT)
                        if pp == 0:
                            outA = psa.tile(
                                [DH + 1, QT], F32, tag="outA", name="outA"
                            )
                            outB = psb.tile(
                                [DH + 1, QT], F32, tag="outB", name="outB"
                            )
                        s_ps = ps2.tile([P, 2, QT], F32, tag="s", name="s_ps")
                        kce, kco = 2 * pp, 2 * pp + 1
                        nc.tensor.matmul(
                            s_ps[:, 0, :],
                            lhsT=kTlow_b[b][:, kce * P : (kce + 1) * P],
                            rhs=qkT_b[b][0:DH, qs],
                            start=True,
                            stop=True,
                        )
                        nc.tensor.matmul(
                            s_ps[:, 1, :],
                            lhsT=qkT_b[b][DH:P, kco * P : (kco + 1) * P],
                            rhs=qhi_b[b][DH:P, qs],
                            start=True,
                            stop=True,
                        )
                        et = p2.tile([P, 2, QT], BF16, tag="et", name="et", bufs=4)
                        nc.scalar.activation(
                            et[:], s_ps[:], mybir.ActivationFunctionType.Exp
                        )
                        if pending is not None:
                            next(pending, None)
                        at = p2.tile(
                            [P, 2, QT], BF16, tag="at", name="at", bufs=LAG + 2
                        )
                        nc.vector.tensor_mul(at[:], et[:], expb_t[pp][:, :, qs])
                        inflight.append((at, pp, b, jq, bji, outA, outB))
                    if len(inflight) > (LAG if gi < total else 0):
                        pat, ppp, pb, pjq, pbji, poutA, poutB = inflight.pop(0)
                        for i in range(2):
                            kc = 2 * ppp + i
                            nc.tensor.matmul(
                                poutA[:],
                                lhsT=v_b[pb][0:DH, kc, 0 : DH + 1],
                                rhs=pat[0:DH, i, :],
                                start=(kc == 0),
                                stop=(kc == KC - 1),
                            )
                            nc.tensor.matmul(
                                poutB[:],
                                lhsT=v_b[pb][DH:P, kc, 0 : DH + 1],
                                rhs=pat[DH:P, i, :],
                                start=(kc == 0),
                                stop=(kc == KC - 1),
                            )
                        if ppp == NPAIR - 1:
                            if pending is not None:
                                for _ in pending:
                                    pass
                            pending = epilogue_tail(
                                pb, pjq, rs64_t[pbji % 2], poutA, poutB
                            )
                    if gi >= total and pending is not None:
                        next(pending, None)
                for _ in pending:
                    pass

    nc.compile()
    return nc


def _host_inputs(x, pos_bias, W_qkv, W_out):
    """Build the per-core input maps (pure data marshalling)."""
    import ml_dtypes

    bf16 = ml_dtypes.bfloat16

    xT = x.transpose(0, 2, 1)                                # [B, DIM, N]
    xT4 = np.ascontiguousarray(
        xT.reshape(B, DC, P, N).transpose(0, 2, 1, 3)
    ).astype(bf16)                                           # [B, P, DC, N]

    # rotary tables in [d, n] layout, deinterleaved (evens then odds);
    # rows 0:32 q-even, 32:64 q-odd, 64:96 k-even, 96:128 k-odd.
    h = DH // 2
    inv_freq = 1.0 / (10000.0 ** (np.arange(0, DH, 2, dtype=np.float64) / DH))
    nn = np.arange(N, dtype=np.float64)
    cos32 = np.cos(inv_freq[:, None] * nn[None, :])          # [32, N]
    sin32 = np.sin(inv_freq[:, None] * nn[None, :])
    cosT = np.tile(cos32, (4, 1)).astype(bf16)               # [128, N]
    sinT = np.tile(sin32, (4, 1)).astype(bf16)

    # pair-swap permutation: rot[e_j] = -in[o_j], rot[o_j] = +in[e_j]
    # within each of the q (0:64) and k (64:128) blocks.
    S = np.zeros((P, P), dtype=np.float32)
    for base in (0, DH):
        for j in range(h):
            S[base + j, base + h + j] = -1.0
            S[base + h + j, base + j] = +1.0
    S_T = np.ascontiguousarray(S.T).astype(bf16)

    # broadcast matrix for the row-sum matmul: out rows 0:64 pick up
    # rs row 0 (rows 1:64 of rs are kept zero)
    ones64 = np.zeros((DH, P), dtype=np.float32)
    ones64[:, 0:DH] = 1.0

    scale = np.float32(DH**-0.5)
    deint = np.concatenate([np.arange(0, DH, 2), np.arange(1, DH, 2)])
    in_maps = []
    for hh in range(HEADS):
        Wq = W_qkv[:, hh * DH : (hh + 1) * DH] * scale
        Wk = W_qkv[:, DIM + hh * DH : DIM + (hh + 1) * DH]
        Wv = W_qkv[:, 2 * DIM + hh * DH : 2 * DIM + (hh + 1) * DH]
        Wqk = np.concatenate([Wq[:, deint], Wk[:, deint]], axis=1)  # [512, 128]
        wqk = np.ascontiguousarray(
            Wqk.reshape(DC, P, P).transpose(1, 0, 2)
        ).astype(bf16)                                       # [P, DC, P]
        wv = np.ascontiguousarray(
            Wv.reshape(DC, P, DH).transpose(1, 0, 2)
        ).astype(bf16)                                       # [P, DC, DH]
        expb = np.exp(pos_bias[hh].T).reshape(KC, P, N).astype(bf16)
        woutw = W_out[hh * DH : (hh + 1) * DH, :].astype(bf16)
        in_maps.append(
            {
                "xT4": xT4,
                "wqk": wqk,
                "wv": wv,
                "st": S_T,
                "cosT": cosT,
                "sinT": sinT,
                "expb": expb,
                "woutw": woutw,
                "ones64": ones64,
                "zeros64": np.zeros((DH, QT), dtype=np.float32),
            }
        )
    return in_maps


def _sim(x, pos_bias, W_qkv, W_out):
    """Numpy mirror of the device algorithm (for marshalling validation)."""
    import ml_dtypes

    bf16 = ml_dtypes.bfloat16
    in_maps = _host_inputs(x, pos_bias, W_qkv, W_out)
    out = np.zeros((B, N, DIM), dtype=np.float32)
    for hh in range(HEADS):
        m = in_maps[hh]
        wqk = m["wqk"].astype(np.float32).transpose(1, 0, 2).reshape(DIM, P)
        wv = m["wv"].astype(np.float32).transpose(1, 0, 2).reshape(DIM, DH)
        cosT = m["cosT"].astype(np.float32)
        sinT = m["sinT"].astype(np.float32)
        S_T = m["st"].astype(np.float32)
        for b in range(B):
            xb = m["xT4"][b].astype(np.float32).transpose(1, 0, 2).reshape(DIM, N)
            qkT = (wqk.T @ xb).astype(bf16).astype(np.float32)      # [128, N]
            rot = S_T.T @ qkT
            qkT = (
                (qkT * cosT).astype(bf16).astype(np.float32)
                + (rot * sinT).astype(bf16).astype(np.float32)
            ).astype(bf16).astype(np.float32)
            v = (xb.T @ wv).astype(bf16).astype(np.float32)         # [N, DH]
            qT, kT = qkT[0:DH], qkT[DH:P]
            s = kT.T @ qT                                           # [k, q]
            et = np.exp(s).astype(bf16).astype(np.float32)
            expb = (
                m["expb"].astype(np.float32).reshape(N, N)
            )                                                       # [k, q]
            at = (et * expb).astype(bf16).astype(np.float32)
            outT = np.concatenate([v, np.ones((N, 1), np.float32)], 1).T @ at
            ho = (
                (outT[0:DH] / outT[DH : DH + 1]).astype(bf16).astype(np.float32)
            )
            wo = ho.T @ m["woutw"].astype(np.float32)               # [N, DIM]
            out[b] += wo.astype(bf16).astype(np.float32)
    return out


def kernel(x, pos_bias, W_qkv, W_out, _trace=False):
    from concourse.bass_utils import run_bass_kernel_spmd

    x = np.asarray(x, dtype=np.float32)
    pos_bias = np.asarray(pos_bias, dtype=np.float32)
    W_qkv = np.asarray(W_qkv, dtype=np.float32)
    W_out = np.asarray(W_out, dtype=np.float32)

    if "nc" not in _CACHE:
        _CACHE["nc"] = _build()
    nc = _CACHE["nc"]

    in_maps = _host_inputs(x, pos_bias, W_qkv, W_out)
    try:
        res = run_bass_kernel_spmd(
            nc, in_maps, core_ids=list(range(HEADS)), trace=_trace
        )
    except ModuleNotFoundError:
        res = run_bass_kernel_spmd(
            nc, in_maps, core_ids=list(range(HEADS)), trace=False
        )
    out = np.zeros((B, N, DIM), dtype=np.float32)
    for rmap in res.results:
        out += np.asarray(rmap["out"], dtype=np.float32)
    if _trace:
        return out, res
    return out


if __name__ == "__main__":
    rng = np.random.default_rng(0)
    x = rng.standard_normal((B, N, DIM), dtype=np.float32)
    pb = rng.standard_normal((HEADS, N, N), dtype=np.float32)
    wq = rng.standard_normal((DIM, 3 * DIM), dtype=np.float32) * DIM**-0.5
    wo = rng.standard_normal((DIM, DIM), dtype=np.float32) * DIM**-0.5
    o = kernel(x, pb, wq, wo)
    print("kernel ran, out std:", o.std())


# revision 40
# speedup vs baseline: 1.1303x; 1.1303x over previous
"""Attention kernel for trn2: B=4, N=2048, DIM=512, HEADS=8, DIM_HEAD=64.

Sharding: head-parallel across 8 cores (core h computes head h for all 4
batches). Each core returns its head's contribution through W_out as a
bf16 partial [4, 2048, 512]; the host sums the 8 partials in fp32.

Per-core pipeline (bf16 data path, fp32 PSUM accumulation):
  phase 1: qkT = (W_qk)^T x^T computed directly in [d, n] layout; rotary
           applied via a +-1 permutation matmul (pair swap) plus 3 DVE
           ops against precomputed cos/sin tables; v in natural [n, d]
           layout with a ones column for row sums.
  phase 2: every matmul is contract-64 and runs 2x via PE row tiling
           (two concurrent 64x128 sub-array tiles at partitions 0:64 /
           64:128):
             S^T[k,q]: even key-chunks on tile (0,0) (k from kTlow,
             q from qkT rows 0:64), odd chunks on tile (64,0) (k from
             qkT rows 64:128, q from a duplicated copy at rows 64:128).
             exp on ACT (1024 wide), attn = et * exp(bias) on DVE (bf16,
             exp(bias) precomputed on host).
             PV: each 128-key chunk split into two 64-key halves run
             concurrently on both tiles into separate accumulators
             outT_A/outT_B (summed in the epilogue).
             W_out: two 128-row chunks run concurrently (ho duplicated
             to partitions 64:128 by SBUF-to-SBUF DMA).
"""

import itertools

import numpy as np

B, N, DIM = 4, 2048, 512
HEADS, DH = 8, 64
P = 128
DC = DIM // P          # 4 input-dim chunks
KC = N // P            # 16 key chunks
QT = 512               # query tile in phase 2
NQT = N // QT          # 4
NPAIR = KC // 2        # 8 key-chunk pairs

_CACHE = {}


def _build():
    import concourse.mybir as mybir
    import concourse.tile as tile
    from concourse import bacc

    F32 = mybir.dt.float32
    F32R = mybir.dt.float32r
    BF16 = mybir.dt.bfloat16

    nc = bacc.Bacc(None, target_bir_lowering=False)

    xT4_d = nc.dram_tensor("xT4", [B, P, DC, N], BF16, kind="ExternalInput")
    wqk_d = nc.dram_tensor("wqk", [P, DC, P], BF16, kind="ExternalInput")
    wv_d = nc.dram_tensor("wv", [P, DC, DH], BF16, kind="ExternalInput")
    st_d = nc.dram_tensor("st", [P, P], BF16, kind="ExternalInput")
    cosT_d = nc.dram_tensor("cosT", [P, N], BF16, kind="ExternalInput")
    sinT_d = nc.dram_tensor("sinT", [P, N], BF16, kind="ExternalInput")
    expb_d = nc.dram_tensor("expb", [KC, P, N], BF16, kind="ExternalInput")
    woutw_d = nc.dram_tensor("woutw", [DH, DIM], BF16, kind="ExternalInput")
    ones64_d = nc.dram_tensor("ones64", [DH, P], F32R, kind="ExternalInput")
    zeros64_d = nc.dram_tensor("zeros64", [DH, QT], F32R, kind="ExternalInput")
    out_d = nc.dram_tensor("out", [B, N, DIM], BF16, kind="ExternalOutput")

    with tile.TileContext(nc) as tc:
        with tc.tile_pool(name="const", bufs=1) as cp:
            wqk_t = cp.tile([P, DC, P], BF16, tag="wqk")
            nc.sync.dma_start(wqk_t[:], wqk_d[:, :, :])
            wv_t = cp.tile([P, DC, DH], BF16, tag="wv")
            nc.sync.dma_start(wv_t[:], wv_d[:, :, :])
            st_t = cp.tile([P, P], BF16, tag="st")
            nc.sync.dma_start(st_t[:], st_d[:, :])
            cosT_t = cp.tile([P, N], BF16, tag="cosT")
            sinT_t = cp.tile([P, N], BF16, tag="sinT")
            nc.sync.dma_start(cosT_t[:], cosT_d[:, :])
            nc.sync.dma_start(sinT_t[:], sinT_d[:, :])
            # W_out rhs duplicated into both partition halves for the
            # row-tiled wout pair
            woutw2_t = cp.tile([P, DIM], BF16, tag="woutw2")
            nc.sync.dma_start(woutw2_t[0:DH, :], woutw_d[:, :])
            nc.sync.dma_start(woutw2_t[DH:P, :], woutw_d[:, :])
            ones64_t = cp.tile([DH, P], F32R, tag="ones64")
            nc.sync.dma_start(ones64_t[:], ones64_d[:, :])
            # row-sum staging: row 0 written per (b, jq); rows 1:64 stay 0
            rs64_t = [
                cp.tile([DH, QT], F32R, tag=f"rs64_{i}", name=f"rs64_{i}")
                for i in range(2)
            ]
            for t in rs64_t:
                nc.sync.dma_start(t[:], zeros64_d[:, :])
            expb_t = [
                cp.tile([P, 2, N], BF16, tag=f"expb{t}", name=f"expb{t}")
                for t in range(NPAIR)
            ]

            def load_expb(ts):
                # staggered through phase 1 so the 8.4MB doesn't contend
                # with the startup-critical x/weight loads
                for t in ts:
                    nc.scalar.dma_start(
                        expb_t[t][:],
                        expb_d[2 * t : 2 * t + 2, :, :].rearrange("k p n -> p k n"),
                    )

            qkT_b = [
                cp.tile([P, N], BF16, tag=f"qkT{b}", name=f"qkT{b}") for b in range(B)
            ]
            kTlow_b = [
                cp.tile([DH, N], BF16, tag=f"kTlow{b}", name=f"kTlow{b}")
                for b in range(B)
            ]
            qhi_b = [
                cp.tile([P, N], BF16, tag=f"qhi{b}", name=f"qhi{b}")
                for b in range(B)
            ]
            v_b = [
                cp.tile([P, KC, DH + 2], BF16, tag=f"v{b}", name=f"v{b}")
                for b in range(B)
            ]

            # ---- phase 1: qkv projection (transposed) + rotary ----
            with (
                tc.tile_pool(name="p1", bufs=2) as p1,
                tc.tile_pool(name="pp1", bufs=2, space="PSUM") as pp1,
            ):
                for b in range(B):
                    xt = p1.tile([P, DC, N], BF16, tag="xt")
                    for mc in range(N // QT):
                        ms = slice(mc * QT, (mc + 1) * QT)
                        nc.sync.dma_start(xt[:, :, ms], xT4_d[b, :, :, ms])
                    for mc in range(N // QT):
                        ms = slice(mc * QT, (mc + 1) * QT)
                        qk_ps = pp1.tile([P, QT], F32, tag="qkps")
                        for dc in range(DC):
                            nc.tensor.matmul(
                                qk_ps[:],
                                lhsT=wqk_t[:, dc, :],
                                rhs=xt[:, dc, ms],
                                start=(dc == 0),
                                stop=(dc == DC - 1),
                            )
                        qk_sb = p1.tile([P, QT], BF16, tag="qksb")
                        nc.scalar.copy(qk_sb[:], qk_ps[:])
                        rot_ps = pp1.tile([P, QT], F32, tag="rotps")
                        nc.tensor.matmul(
                            rot_ps[:], lhsT=st_t[:], rhs=qk_sb[:],
                            start=True, stop=True,
                        )
                        # v for the four 128-row chunks of this column block;
                        # interleaved here so their weight loads hide under
                        # the long qk/rot matmuls
                        v_ps = pp1.tile([P, 4, DH], F32, tag="vps")
                        for i in range(4):
                            ch = mc * 4 + i
                            for dc in range(DC):
                                nc.tensor.matmul(
                                    v_ps[:, i, :],
                                    lhsT=xt[:, dc, ch * P : (ch + 1) * P],
                                    rhs=wv_t[:, dc, :],
                                    start=(dc == 0),
                                    stop=(dc == DC - 1),
                                )
                        a_t = p1.tile([P, QT], BF16, tag="a")
                        nc.vector.tensor_mul(a_t[:], qk_sb[:], cosT_t[:, ms])
                        rb_t = p1.tile([P, QT], BF16, tag="rb")
                        nc.vector.tensor_mul(rb_t[:], rot_ps[:], sinT_t[:, ms])
                        nc.vector.tensor_add(qkT_b[b][:, ms], a_t[:], rb_t[:])
                        # matmul operands must share a base partition: stage
                        # k at partitions 0:64 and a q copy at 64:128
                        nc.sync.dma_start(kTlow_b[b][:, ms], qkT_b[b][DH:P, ms])
                        nc.sync.dma_start(
                            qhi_b[b][DH:P, ms], qkT_b[b][0:DH, ms]
                        )
                        nc.scalar.copy(
                            v_b[b][:, mc * 4 : (mc + 1) * 4, 0:DH], v_ps[:]
                        )
                    nc.vector.memset(v_b[b][:, :, DH : DH + 1], 1.0)
                    load_expb(range(2 * b, 2 * b + 2))

            # ---- phase 2: attention (all matmuls 64x128 row-tiled) ----
            with (
                tc.tile_pool(name="p2", bufs=3) as p2,
                tc.tile_pool(name="ps2", bufs=2, space="PSUM") as ps2,
                tc.tile_pool(name="psa", bufs=1, space="PSUM") as psa,
                tc.tile_pool(name="psb", bufs=1, space="PSUM") as psb,
                tc.tile_pool(name="psw", bufs=2, space="PSUM") as psw,
            ):

                def epilogue_tail(b, jq, rs, outA, outB):
                    """Epilogue steps, one dependency hop per chunk so no
                    engine queue head-of-line blocks ahead of the next
                    block's exp/mult work. Both accumulators are drained
                    concurrently (ACT || DVE) so the next block's first PV
                    pair is released as fast as possible."""
                    outA_sb = p2.tile(
                        [DH + 1, QT], F32, tag="outAsb", name="outA_sb", bufs=2
                    )
                    nc.scalar.copy(outA_sb[:], outA[:])
                    outB_sb = p2.tile(
                        [DH + 1, QT], F32, tag="outBsb", name="outB_sb", bufs=2
                    )
                    nc.vector.tensor_copy(outB_sb[:], outB[:])
                    yield
                    with nc.allow_low_precision(reason="f32r row-sum staging"):
                        nc.vector.tensor_add(
                            rs[0:1, :],
                            outA_sb[DH : DH + 1, :],
                            outB_sb[DH : DH + 1, :],
                        )
                    hsum = p2.tile([DH, QT], F32, tag="hsum", name="hsum", bufs=2)
                    nc.vector.tensor_add(
                        hsum[:], outA_sb[0:DH, :], outB_sb[0:DH, :]
                    )
                    yield
                    bc_ps = psw.tile([P, QT], F32, tag="wo", name="bc_ps")
                    nc.tensor.matmul(
                        bc_ps[:], lhsT=ones64_t[:], rhs=rs[:],
                        start=True, stop=True,
                    )
                    yield
                    bcr_sb = p2.tile([DH, QT], F32, tag="bcr", name="bcr_sb", bufs=2)
                    nc.vector.reciprocal(bcr_sb[:], bc_ps[0:DH, :])
                    yield
                    ho_t = p2.tile([P, QT], BF16, tag="ho", name="ho_t", bufs=2)
                    nc.vector.tensor_mul(ho_t[0:DH, :], hsum[:], bcr_sb[:])
                    yield
                    nc.sync.dma_start(ho_t[DH:P, :], ho_t[0:DH, :])
                    yield
                    for sqp in range(2):
                        wo_lo = psw.tile([P, DIM], F32, tag="wo", name="wo_lo")
                        nc.tensor.matmul(
                            wo_lo[:],
                            lhsT=ho_t[0:DH, (2 * sqp) * P : (2 * sqp + 1) * P],
                            rhs=woutw2_t[0:DH, :],
                            start=True,
                            stop=True,
                        )
                        wo_hi = psw.tile([P, DIM], F32, tag="wo", name="wo_hi")
                        nc.tensor.matmul(
                            wo_hi[:],
                            lhsT=ho_t[DH:P, (2 * sqp + 1) * P : (2 * sqp + 2) * P],
                            rhs=woutw2_t[DH:P, :],
                            start=True,
                            stop=True,
                        )
                        yield
                        for half, wo_ps in enumerate((wo_lo, wo_hi)):
                            ob = p2.tile([P, DIM], BF16, tag="ob", name="ob")
                            if half == 0:
                                nc.scalar.copy(ob[:], wo_ps[:])
                            else:
                                nc.vector.tensor_copy(ob[:], wo_ps[:])
                            row0 = jq * QT + (2 * sqp + half) * P
                            nc.sync.dma_start(out_d[b, row0 : row0 + P, :], ob[:])
                        yield

                # one continuous software-pipelined stream over all
                # (b, jq, pair) steps: PV lags S/exp/mult by LAG pairs even
                # across (b, jq) boundaries, so the S->exp->mult->PV chain
                # latency (~3us) amortizes instead of stalling each pair
                LAG = 3
                pending = None
                inflight = []
                outA = outB = None
                total = B * NQT * NPAIR
                for gi in range(total + LAG):
                    if gi < total:
                        bji, pp = divmod(gi, NPAIR)
                        b, jq = divmod(bji, NQT)
                        qs = slice(jq * QT, (jq + 1) * QT)
                        if pp == 0:
                            outA = psa.tile(
                                [DH + 1, QT], F32, tag="outA", name="outA"
                            )
                            outB = psb.tile(
                                [DH + 1, QT], F32, tag="outB", name="outB"
                            )
                        s_ps = ps2.tile([P, 2, QT], F32, tag="s", name="s_ps")
                        kce, kco = 2 * pp, 2 * pp + 1
                        nc.tensor.matmul(
                            s_ps[:, 0, :],
                            lhsT=kTlow_b[b][:, kce * P : (kce + 1) * P],
                            rhs=qkT_b[b][0:DH, qs],
                            start=True,
                            stop=True,
                        )
                        nc.tensor.matmul(
                            s_ps[:, 1, :],
                            lhsT=qkT_b[b][DH:P, kco * P : (kco + 1) * P],
                            rhs=qhi_b[b][DH:P, qs],
                            start=True,
                            stop=True,
                        )
                        et = p2.tile([P, 2, QT], BF16, tag="et", name="et", bufs=4)
                        nc.scalar.activation(
                            et[:], s_ps[:], mybir.ActivationFunctionType.Exp
                        )
                        if pending is not None:
                            next(pending, None)
                        at = p2.tile(
                            [P, 2, QT], BF16, tag="at", name="at", bufs=LAG + 2
                        )
                        nc.vector.tensor_mul(at[:], et[:], expb_t[pp][:, :, qs])
                        inflight.append((at, pp, b, jq, bji, outA, outB))
                    if len(inflight) > (LAG if gi < total else 0):
                        pat, ppp, pb, pjq, pbji, poutA, poutB = inflight.pop(0)
                        for i in range(2):
                            kc = 2 * ppp + i
                            nc.tensor.matmul(
                                poutA[:],
                                lhsT=v_b[pb][0:DH, kc, 0 : DH + 1],
                                rhs=pat[0:DH, i, :],
                                start=(kc == 0),
                                stop=(kc == KC - 1),
                            )
                            nc.tensor.matmul(
                                poutB[:],
                                lhsT=v_b[pb][DH:P, kc, 0 : DH + 1],
                                rhs=pat[DH:P, i, :],
                                start=(kc == 0),
                                stop=(kc == KC - 1),
                            )
                        if ppp == NPAIR - 1:
                            if pending is not None:
                                for _ in pending:
                                    pass
                            pending = epilogue_tail(
                                pb, pjq, rs64_t[pbji % 2], poutA, poutB
                            )
                    if gi >= total and pending is not None:
                        next(pending, None)
                for _ in pending:
                    pass

    nc.compile()
    return nc


def _host_inputs(x, pos_bias, W_qkv, W_out):
    """Build the per-core input maps (pure data marshalling)."""
    import ml_dtypes

    bf16 = ml_dtypes.bfloat16

    xT = x.transpose(0, 2, 1)                                # [B, DIM, N]
    xT4 = np.ascontiguousarray(
        xT.reshape(B, DC, P, N).transpose(0, 2, 1, 3)
    ).astype(bf16)                                           # [B, P, DC, N]

    # rotary tables in [d, n] layout, deinterleaved (evens then odds);
    # rows 0:32 q-even, 32:64 q-odd, 64:96 k-even, 96:128 k-odd.
    h = DH // 2
    inv_freq = 1.0 / (10000.0 ** (np.arange(0, DH, 2, dtype=np.float64) / DH))
    nn = np.arange(N, dtype=np.float64)
    cos32 = np.cos(inv_freq[:, None] * nn[None, :])          # [32, N]
    sin32 = np.sin(inv_freq[:, None] * nn[None, :])
    cosT = np.tile(cos32, (4, 1)).astype(bf16)               # [128, N]
    sinT = np.tile(sin32, (4, 1)).astype(bf16)

    # pair-swap permutation: rot[e_j] = -in[o_j], rot[o_j] = +in[e_j]
    # within each of the q (0:64) and k (64:128) blocks.
    S = np.zeros((P, P), dtype=np.float32)
    for base in (0, DH):
        for j in range(h):
            S[base + j, base + h + j] = -1.0
            S[base + h + j, base + j] = +1.0
    S_T = np.ascontiguousarray(S.T).astype(bf16)

    # broadcast matrix for the row-sum matmul: out rows 0:64 pick up
    # rs row 0 (rows 1:64 of rs are kept zero)
    ones64 = np.zeros((DH, P), dtype=np.float32)
    ones64[:, 0:DH] = 1.0

    scale = np.float32(DH**-0.5)
    deint = np.concatenate([np.arange(0, DH, 2), np.arange(1, DH, 2)])
    in_maps = []
    for hh in range(HEADS):
        Wq = W_qkv[:, hh * DH : (hh + 1) * DH] * scale
        Wk = W_qkv[:, DIM + hh * DH : DIM + (hh + 1) * DH]
        Wv = W_qkv[:, 2 * DIM + hh * DH : 2 * DIM + (hh + 1) * DH]
        Wqk = np.concatenate([Wq[:, deint], Wk[:, deint]], axis=1)  # [512, 128]
        wqk = np.ascontiguousarray(
            Wqk.reshape(DC, P, P).transpose(1, 0, 2)
        ).astype(bf16)                                       # [P, DC, P]
        wv = np.ascontiguousarray(
            Wv.reshape(DC, P, DH).transpose(1, 0, 2)
        ).astype(bf16)                                       # [P, DC, DH]
        expb = np.exp(pos_bias[hh].T).reshape(KC, P, N).astype(bf16)
        woutw = W_out[hh * DH : (hh + 1) * DH, :].astype(bf16)
        in_maps.append(
            {
                "xT4": xT4,
                "wqk": wqk,
                "wv": wv,
                "st": S_T,
                "cosT": cosT,
                "sinT": sinT,
                "expb": expb,
                "woutw": woutw,
                "ones64": ones64,
                "zeros64": np.zeros((DH, QT), dtype=np.float32),
            }
        )
    return in_maps


def _sim(x, pos_bias, W_qkv, W_out):
    """Numpy mirror of the device algorithm (for marshalling validation)."""
    import ml_dtypes

    bf16 = ml_dtypes.bfloat16
    in_maps = _host_inputs(x, pos_bias, W_qkv, W_out)
    out = np.zeros((B, N, DIM), dtype=np.float32)
    for hh in range(HEADS):
        m = in_maps[hh]
        wqk = m["wqk"].astype(np.float32).transpose(1, 0, 2).reshape(DIM, P)
        wv = m["wv"].astype(np.float32).transpose(1, 0, 2).reshape(DIM, DH)
        cosT = m["cosT"].astype(np.float32)
        sinT = m["sinT"].astype(np.float32)
        S_T = m["st"].astype(np.float32)
        for b in range(B):
            xb = m["xT4"][b].astype(np.float32).transpose(1, 0, 2).reshape(DIM, N)
            qkT = (wqk.T @ xb).astype(bf16).astype(np.float32)      # [128, N]
            rot = S_T.T @ qkT
            qkT = (
                (qkT * cosT).astype(bf16).astype(np.float32)
                + (rot * sinT).astype(bf16).astype(np.float32)
            ).astype(bf16).astype(np.float32)
            v = (xb.T @ wv).astype(bf16).astype(np.float32)         # [N, DH]
            qT, kT = qkT[0:DH], qkT[DH:P]
            s = kT.T @ qT                                           # [k, q]
            et = np.exp(s).astype(bf16).astype(np.float32)
            expb = (
                m["expb"].astype(np.float32).reshape(N, N)
            )                                                       # [k, q]
            at = (et * expb).astype(bf16).astype(np.float32)
            outT = np.concatenate([v, np.ones((N, 1), np.float32)], 1).T @ at
            ho = (
                (outT[0:DH] / outT[DH : DH + 1]).astype(bf16).astype(np.float32)
            )
            wo = ho.T @ m["woutw"].astype(np.float32)               # [N, DIM]
            out[b] += wo.astype(bf16).astype(np.float32)
    return out


def kernel(x, pos_bias, W_qkv, W_out, _trace=False):
    from concourse.bass_utils import run_bass_kernel_spmd

    x = np.asarray(x, dtype=np.float32)
    pos_bias = np.asarray(pos_bias, dtype=np.float32)
    W_qkv = np.asarray(W_qkv, dtype=np.float32)
    W_out = np.asarray(W_out, dtype=np.float32)

    if "nc" not in _CACHE:
        _CACHE["nc"] = _build()
    nc = _CACHE["nc"]

    in_maps = _host_inputs(x, pos_bias, W_qkv, W_out)
    try:
        res = run_bass_kernel_spmd(
            nc, in_maps, core_ids=list(range(HEADS)), trace=_trace
        )
    except ModuleNotFoundError:
        res = run_bass_kernel_spmd(
            nc, in_maps, core_ids=list(range(HEADS)), trace=False
        )
    out = np.zeros((B, N, DIM), dtype=np.float32)
    for rmap in res.results:
        out += np.asarray(rmap["out"], dtype=np.float32)
    if _trace:
        return out, res
    return out


if __name__ == "__main__":
    rng = np.random.default_rng(0)
    x = rng.standard_normal((B, N, DIM), dtype=np.float32)
    pb = rng.standard_normal((HEADS, N, N), dtype=np.float32)
    wq = rng.standard_normal((DIM, 3 * DIM), dtype=np.float32) * DIM**-0.5
    wo = rng.standard_normal((DIM, DIM), dtype=np.float32) * DIM**-0.5
    o = kernel(x, pb, wq, wo)
    print("kernel ran, out std:", o.std())


# revision 41
# speedup vs baseline: 1.1585x; 1.0249x over previous
"""Attention kernel for trn2: B=4, N=2048, DIM=512, HEADS=8, DIM_HEAD=64.

Sharding: head-parallel across 8 cores (core h computes head h for all 4
batches). Each core returns its head's contribution through W_out as a
bf16 partial [4, 2048, 512]; the host sums the 8 partials in fp32.

Per-core pipeline (bf16 data path, fp32 PSUM accumulation):
  phase 1: qkT = (W_qk)^T x^T computed directly in [d, n] layout; rotary
           applied via a +-1 permutation matmul (pair swap) plus 3 DVE
           ops against precomputed cos/sin tables; v in natural [n, d]
           layout with a ones column for row sums.
  phase 2: every matmul is contract-64 and runs 2x via PE row tiling
           (two concurrent 64x128 sub-array tiles at partitions 0:64 /
           64:128):
             S^T[k,q]: even key-chunks on tile (0,0) (k from kTlow,
             q from qkT rows 0:64), odd chunks on tile (64,0) (k from
             qkT rows 64:128, q from a duplicated copy at rows 64:128).
             exp on ACT (1024 wide), attn = et * exp(bias) on DVE (bf16,
             exp(bias) precomputed on host).
             PV: each 128-key chunk split into two 64-key halves run
             concurrently on both tiles into separate accumulators
             outT_A/outT_B (summed in the epilogue).
             W_out: two 128-row chunks run concurrently (ho duplicated
             to partitions 64:128 by SBUF-to-SBUF DMA).
"""

import itertools

import numpy as np

B, N, DIM = 4, 2048, 512
HEADS, DH = 8, 64
P = 128
DC = DIM // P          # 4 input-dim chunks
KC = N // P            # 16 key chunks
QT = 512               # query tile in phase 2
NQT = N // QT          # 4
NPAIR = KC // 2        # 8 key-chunk pairs

_CACHE = {}


def _build():
    import concourse.mybir as mybir
    import concourse.tile as tile
    from concourse import bacc

    F32 = mybir.dt.float32
    F32R = mybir.dt.float32r
    BF16 = mybir.dt.bfloat16

    nc = bacc.Bacc(None, target_bir_lowering=False)

    xT4_d = nc.dram_tensor("xT4", [B, P, DC, N], BF16, kind="ExternalInput")
    wqk_d = nc.dram_tensor("wqk", [P, DC, P], BF16, kind="ExternalInput")
    wv_d = nc.dram_tensor("wv", [P, DC, DH], BF16, kind="ExternalInput")
    st_d = nc.dram_tensor("st", [P, P], BF16, kind="ExternalInput")
    cosT_d = nc.dram_tensor("cosT", [P, N], BF16, kind="ExternalInput")
    sinT_d = nc.dram_tensor("sinT", [P, N], BF16, kind="ExternalInput")
    expb_d = nc.dram_tensor("expb", [KC, P, N], BF16, kind="ExternalInput")
    woutw_d = nc.dram_tensor("woutw", [DH, DIM], BF16, kind="ExternalInput")
    ones64_d = nc.dram_tensor("ones64", [DH, P], F32R, kind="ExternalInput")
    zeros64_d = nc.dram_tensor("zeros64", [DH, QT], F32R, kind="ExternalInput")
    out_d = nc.dram_tensor("out", [B, N, DIM], BF16, kind="ExternalOutput")

    with tile.TileContext(nc) as tc:
        with tc.tile_pool(name="const", bufs=1) as cp:
            wqk_t = cp.tile([P, DC, P], BF16, tag="wqk")
            nc.sync.dma_start(wqk_t[:], wqk_d[:, :, :])
            wv_t = cp.tile([P, DC, DH], BF16, tag="wv")
            nc.sync.dma_start(wv_t[:], wv_d[:, :, :])
            st_t = cp.tile([P, P], BF16, tag="st")
            nc.sync.dma_start(st_t[:], st_d[:, :])
            cosT_t = cp.tile([P, N], BF16, tag="cosT")
            sinT_t = cp.tile([P, N], BF16, tag="sinT")
            woutw2_t = cp.tile([P, DIM], BF16, tag="woutw2")
            ones64_t = cp.tile([DH, P], F32R, tag="ones64")
            rs64_t = [
                cp.tile([DH, QT], F32R, tag=f"rs64_{i}", name=f"rs64_{i}")
                for i in range(2)
            ]

            def load_late_consts():
                # deferred behind the startup-critical x(b=0) load
                nc.sync.dma_start(cosT_t[:], cosT_d[:, :])
                nc.sync.dma_start(sinT_t[:], sinT_d[:, :])
                # W_out rhs duplicated into both partition halves for the
                # row-tiled wout pair
                nc.sync.dma_start(woutw2_t[0:DH, :], woutw_d[:, :])
                nc.sync.dma_start(woutw2_t[DH:P, :], woutw_d[:, :])
                nc.sync.dma_start(ones64_t[:], ones64_d[:, :])
                # row-sum staging: row 0 written per (b, jq); rows 1:64
                # stay 0
                for t in rs64_t:
                    nc.sync.dma_start(t[:], zeros64_d[:, :])
            expb_t = [
                cp.tile([P, 2, N], BF16, tag=f"expb{t}", name=f"expb{t}")
                for t in range(NPAIR)
            ]

            def load_expb(ts):
                # staggered through phase 1 so the 8.4MB doesn't contend
                # with the startup-critical x/weight loads
                for t in ts:
                    nc.scalar.dma_start(
                        expb_t[t][:],
                        expb_d[2 * t : 2 * t + 2, :, :].rearrange("k p n -> p k n"),
                    )

            qkT_b = [
                cp.tile([P, N], BF16, tag=f"qkT{b}", name=f"qkT{b}") for b in range(B)
            ]
            kTlow_b = [
                cp.tile([DH, N], BF16, tag=f"kTlow{b}", name=f"kTlow{b}")
                for b in range(B)
            ]
            qhi_b = [
                cp.tile([P, N], BF16, tag=f"qhi{b}", name=f"qhi{b}")
                for b in range(B)
            ]
            v_b = [
                cp.tile([P, KC, DH + 2], BF16, tag=f"v{b}", name=f"v{b}")
                for b in range(B)
            ]

            # ---- phase 1: qkv projection (transposed) + rotary ----
            with (
                tc.tile_pool(name="p1", bufs=2) as p1,
                tc.tile_pool(name="pp1", bufs=2, space="PSUM") as pp1,
            ):
                def load_xt(b):
                    xt = p1.tile([P, DC, N], BF16, tag="xt", name=f"xt{b}")
                    for mc in range(N // QT):
                        ms = slice(mc * QT, (mc + 1) * QT)
                        nc.sync.dma_start(xt[:, :, ms], xT4_d[b, :, :, ms])
                    return xt

                xt_q = [load_xt(0)]
                load_late_consts()
                xt_q.append(load_xt(1))
                for b in range(B):
                    xt = xt_q[b]
                    for mc in range(N // QT):
                        ms = slice(mc * QT, (mc + 1) * QT)
                        qk_ps = pp1.tile([P, QT], F32, tag="qkps")
                        for dc in range(DC):
                            nc.tensor.matmul(
                                qk_ps[:],
                                lhsT=wqk_t[:, dc, :],
                                rhs=xt[:, dc, ms],
                                start=(dc == 0),
                                stop=(dc == DC - 1),
                            )
                        qk_sb = p1.tile([P, QT], BF16, tag="qksb")
                        nc.scalar.copy(qk_sb[:], qk_ps[:])
                        rot_ps = pp1.tile([P, QT], F32, tag="rotps")
                        nc.tensor.matmul(
                            rot_ps[:], lhsT=st_t[:], rhs=qk_sb[:],
                            start=True, stop=True,
                        )
                        # v for the four 128-row chunks of this column block;
                        # interleaved here so their weight loads hide under
                        # the long qk/rot matmuls
                        v_ps = pp1.tile([P, 4, DH], F32, tag="vps")
                        for i in range(4):
                            ch = mc * 4 + i
                            for dc in range(DC):
                                nc.tensor.matmul(
                                    v_ps[:, i, :],
                                    lhsT=xt[:, dc, ch * P : (ch + 1) * P],
                                    rhs=wv_t[:, dc, :],
                                    start=(dc == 0),
                                    stop=(dc == DC - 1),
                                )
                        a_t = p1.tile([P, QT], BF16, tag="a")
                        nc.vector.tensor_mul(a_t[:], qk_sb[:], cosT_t[:, ms])
                        rb_t = p1.tile([P, QT], BF16, tag="rb")
                        nc.vector.tensor_mul(rb_t[:], rot_ps[:], sinT_t[:, ms])
                        nc.vector.tensor_add(qkT_b[b][:, ms], a_t[:], rb_t[:])
                        # matmul operands must share a base partition: stage
                        # k at partitions 0:64 and a q copy at 64:128
                        nc.sync.dma_start(kTlow_b[b][:, ms], qkT_b[b][DH:P, ms])
                        nc.sync.dma_start(
                            qhi_b[b][DH:P, ms], qkT_b[b][0:DH, ms]
                        )
                        nc.scalar.copy(
                            v_b[b][:, mc * 4 : (mc + 1) * 4, 0:DH], v_ps[:]
                        )
                    nc.vector.memset(v_b[b][:, :, DH : DH + 1], 1.0)
                    load_expb(range(2 * b, 2 * b + 2))
                    if b + 2 < B:
                        xt_q.append(load_xt(b + 2))

            # ---- phase 2: attention (all matmuls 64x128 row-tiled) ----
            with (
                tc.tile_pool(name="p2", bufs=3) as p2,
                tc.tile_pool(name="ps2", bufs=2, space="PSUM") as ps2,
                tc.tile_pool(name="psa", bufs=1, space="PSUM") as psa,
                tc.tile_pool(name="psb", bufs=1, space="PSUM") as psb,
                tc.tile_pool(name="psw", bufs=2, space="PSUM") as psw,
            ):

                def epilogue_tail(b, jq, rs, outA, outB):
                    """Epilogue steps, one dependency hop per chunk so no
                    engine queue head-of-line blocks ahead of the next
                    block's exp/mult work. Both accumulators are drained
                    concurrently (ACT || DVE) so the next block's first PV
                    pair is released as fast as possible."""
                    outA_sb = p2.tile(
                        [DH + 1, QT], F32, tag="outAsb", name="outA_sb", bufs=2
                    )
                    nc.scalar.copy(outA_sb[:], outA[:])
                    outB_sb = p2.tile(
                        [DH + 1, QT], F32, tag="outBsb", name="outB_sb", bufs=2
                    )
                    nc.vector.tensor_copy(outB_sb[:], outB[:])
                    yield
                    with nc.allow_low_precision(reason="f32r row-sum staging"):
                        nc.vector.tensor_add(
                            rs[0:1, :],
                            outA_sb[DH : DH + 1, :],
                            outB_sb[DH : DH + 1, :],
                        )
                    hsum = p2.tile([DH, QT], F32, tag="hsum", name="hsum", bufs=2)
                    nc.vector.tensor_add(
                        hsum[:], outA_sb[0:DH, :], outB_sb[0:DH, :]
                    )
                    yield
                    bc_ps = psw.tile([P, QT], F32, tag="wo", name="bc_ps")
                    nc.tensor.matmul(
                        bc_ps[:], lhsT=ones64_t[:], rhs=rs[:],
                        start=True, stop=True,
                    )
                    yield
                    bcr_sb = p2.tile([DH, QT], F32, tag="bcr", name="bcr_sb", bufs=2)
                    nc.vector.reciprocal(bcr_sb[:], bc_ps[0:DH, :])
                    yield
                    ho_t = p2.tile([P, QT], BF16, tag="ho", name="ho_t", bufs=2)
                    nc.vector.tensor_mul(ho_t[0:DH, :], hsum[:], bcr_sb[:])
                    yield
                    nc.sync.dma_start(ho_t[DH:P, :], ho_t[0:DH, :])
                    yield
                    for sqp in range(2):
                        wo_lo = psw.tile([P, DIM], F32, tag="wo", name="wo_lo")
                        nc.tensor.matmul(
                            wo_lo[:],
                            lhsT=ho_t[0:DH, (2 * sqp) * P : (2 * sqp + 1) * P],
                            rhs=woutw2_t[0:DH, :],
                            start=True,
                            stop=True,
                        )
                        wo_hi = psw.tile([P, DIM], F32, tag="wo", name="wo_hi")
                        nc.tensor.matmul(
                            wo_hi[:],
                            lhsT=ho_t[DH:P, (2 * sqp + 1) * P : (2 * sqp + 2) * P],
                            rhs=woutw2_t[DH:P, :],
                            start=True,
                            stop=True,
                        )
                        yield
                        for half, wo_ps in enumerate((wo_lo, wo_hi)):
                            ob = p2.tile([P, DIM], BF16, tag="ob", name="ob")
                            if half == 0:
                                nc.scalar.copy(ob[:], wo_ps[:])
                            else:
                                nc.vector.tensor_copy(ob[:], wo_ps[:])
                            row0 = jq * QT + (2 * sqp + half) * P
                            nc.sync.dma_start(out_d[b, row0 : row0 + P, :], ob[:])
                        yield

                # one continuous software-pipelined stream over all
                # (b, jq, pair) steps: PV lags S/exp/mult by LAG pairs even
                # across (b, jq) boundaries, so the S->exp->mult->PV chain
                # latency (~3us) amortizes instead of stalling each pair
                LAG = 3
                pending = None
                inflight = []
                outA = outB = None
                total = B * NQT * NPAIR
                for gi in range(total + LAG):
                    if gi < total:
                        bji, pp = divmod(gi, NPAIR)
                        b, jq = divmod(bji, NQT)
                        qs = slice(jq * QT, (jq + 1) * QT)
                        if pp == 0:
                            outA = psa.tile(
                                [DH + 1, QT], F32, tag="outA", name="outA"
                            )
                            outB = psb.tile(
                                [DH + 1, QT], F32, tag="outB", name="outB"
                            )
                        s_ps = ps2.tile([P, 2, QT], F32, tag="s", name="s_ps")
                        kce, kco = 2 * pp, 2 * pp + 1
                        nc.tensor.matmul(
                            s_ps[:, 0, :],
                            lhsT=kTlow_b[b][:, kce * P : (kce + 1) * P],
                            rhs=qkT_b[b][0:DH, qs],
                            start=True,
                            stop=True,
                        )
                        nc.tensor.matmul(
                            s_ps[:, 1, :],
                            lhsT=qkT_b[b][DH:P, kco * P : (kco + 1) * P],
                            rhs=qhi_b[b][DH:P, qs],
                            start=True,
                            stop=True,
                        )
                        et = p2.tile([P, 2, QT], BF16, tag="et", name="et", bufs=4)
                        nc.scalar.activation(
                            et[:], s_ps[:], mybir.ActivationFunctionType.Exp
                        )
                        if pending is not None:
                            next(pending, None)
                        at = p2.tile(
                            [P, 2, QT], BF16, tag="at", name="at", bufs=LAG + 2
                        )
                        nc.vector.tensor_mul(at[:], et[:], expb_t[pp][:, :, qs])
                        inflight.append((at, pp, b, jq, bji, outA, outB))
                    if len(inflight) > (LAG if gi < total else 0):
                        pat, ppp, pb, pjq, pbji, poutA, poutB = inflight.pop(0)
                        for i in range(2):
                            kc = 2 * ppp + i
                            nc.tensor.matmul(
                                poutA[:],
                                lhsT=v_b[pb][0:DH, kc, 0 : DH + 1],
                                rhs=pat[0:DH, i, :],
                                start=(kc == 0),
                                stop=(kc == KC - 1),
                            )
                            nc.tensor.matmul(
                                poutB[:],
                                lhsT=v_b[pb][DH:P, kc, 0 : DH + 1],
                                rhs=pat[DH:P, i, :],
                                start=(kc == 0),
                                stop=(kc == KC - 1),
                            )
                        if ppp == NPAIR - 1:
                            if pending is not None:
                                for _ in pending:
                                    pass
                            pending = epilogue_tail(
                                pb, pjq, rs64_t[pbji % 2], poutA, poutB
                            )
                    if gi >= total and pending is not None:
                        next(pending, None)
                for _ in pending:
                    pass

    nc.compile()
    return nc


def _host_inputs(x, pos_bias, W_qkv, W_out):
    """Build the per-core input maps (pure data marshalling)."""
    import ml_dtypes

    bf16 = ml_dtypes.bfloat16

    xT = x.transpose(0, 2, 1)                                # [B, DIM, N]
    xT4 = np.ascontiguousarray(
        xT.reshape(B, DC, P, N).transpose(0, 2, 1, 3)
    ).astype(bf16)                                           # [B, P, DC, N]

    # rotary tables in [d, n] layout, deinterleaved (evens then odds);
    # rows 0:32 q-even, 32:64 q-odd, 64:96 k-even, 96:128 k-odd.
    h = DH // 2
    inv_freq = 1.0 / (10000.0 ** (np.arange(0, DH, 2, dtype=np.float64) / DH))
    nn = np.arange(N, dtype=np.float64)
    cos32 = np.cos(inv_freq[:, None] * nn[None, :])          # [32, N]
    sin32 = np.sin(inv_freq[:, None] * nn[None, :])
    cosT = np.tile(cos32, (4, 1)).astype(bf16)               # [128, N]
    sinT = np.tile(sin32, (4, 1)).astype(bf16)

    # pair-swap permutation: rot[e_j] = -in[o_j], rot[o_j] = +in[e_j]
    # within each of the q (0:64) and k (64:128) blocks.
    S = np.zeros((P, P), dtype=np.float32)
    for base in (0, DH):
        for j in range(h):
            S[base + j, base + h + j] = -1.0
            S[base + h + j, base + j] = +1.0
    S_T = np.ascontiguousarray(S.T).astype(bf16)

    # broadcast matrix for the row-sum matmul: out rows 0:64 pick up
    # rs row 0 (rows 1:64 of rs are kept zero)
    ones64 = np.zeros((DH, P), dtype=np.float32)
    ones64[:, 0:DH] = 1.0

    scale = np.float32(DH**-0.5)
    deint = np.concatenate([np.arange(0, DH, 2), np.arange(1, DH, 2)])
    in_maps = []
    for hh in range(HEADS):
        Wq = W_qkv[:, hh * DH : (hh + 1) * DH] * scale
        Wk = W_qkv[:, DIM + hh * DH : DIM + (hh + 1) * DH]
        Wv = W_qkv[:, 2 * DIM + hh * DH : 2 * DIM + (hh + 1) * DH]
        Wqk = np.concatenate([Wq[:, deint], Wk[:, deint]], axis=1)  # [512, 128]
        wqk = np.ascontiguousarray(
            Wqk.reshape(DC, P, P).transpose(1, 0, 2)
        ).astype(bf16)                                       # [P, DC, P]
        wv = np.ascontiguousarray(
            Wv.reshape(DC, P, DH).transpose(1, 0, 2)
        ).astype(bf16)                                       # [P, DC, DH]
        expb = np.exp(pos_bias[hh].T).reshape(KC, P, N).astype(bf16)
        woutw = W_out[hh * DH : (hh + 1) * DH, :].astype(bf16)
        in_maps.append(
            {
                "xT4": xT4,
                "wqk": wqk,
                "wv": wv,
                "st": S_T,
                "cosT": cosT,
                "sinT": sinT,
                "expb": expb,
                "woutw": woutw,
                "ones64": ones64,
                "zeros64": np.zeros((DH, QT), dtype=np.float32),
            }
        )
    return in_maps


def _sim(x, pos_bias, W_qkv, W_out):
    """Numpy mirror of the device algorithm (for marshalling validation)."""
    import ml_dtypes

    bf16 = ml_dtypes.bfloat16
    in_maps = _host_inputs(x, pos_bias, W_qkv, W_out)
    out = np.zeros((B, N, DIM), dtype=np.float32)
    for hh in range(HEADS):
        m = in_maps[hh]
        wqk = m["wqk"].astype(np.float32).transpose(1, 0, 2).reshape(DIM, P)
        wv = m["wv"].astype(np.float32).transpose(1, 0, 2).reshape(DIM, DH)
        cosT = m["cosT"].astype(np.float32)
        sinT = m["sinT"].astype(np.float32)
        S_T = m["st"].astype(np.float32)
        for b in range(B):
            xb = m["xT4"][b].astype(np.float32).transpose(1, 0, 2).reshape(DIM, N)
            qkT = (wqk.T @ xb).astype(bf16).astype(np.float32)      # [128, N]
            rot = S_T.T @ qkT
            qkT = (
                (qkT * cosT).astype(bf16).astype(np.float32)
                + (rot * sinT).astype(bf16).astype(np.float32)
            ).astype(bf16).astype(np.float32)
            v = (xb.T @ wv).astype(bf16).astype(np.float32)         # [N, DH]
            qT, kT = qkT[0:DH], qkT[DH:P]
            s = kT.T @ qT                                           # [k, q]
            et = np.exp(s).astype(bf16).astype(np.float32)
            expb = (
                m["expb"].astype(np.float32).reshape(N, N)
            )                                                       # [k, q]
            at = (et * expb).astype(bf16).astype(np.float32)
            outT = np.concatenate([v, np.ones((N, 1), np.float32)], 1).T @ at
            ho = (
                (outT[0:DH] / outT[DH : DH + 1]).astype(bf16).astype(np.float32)
            )
            wo = ho.T @ m["woutw"].astype(np.float32)               # [N, DIM]
            out[b] += wo.astype(bf16).astype(np.float32)
    return out


def kernel(x, pos_bias, W_qkv, W_out, _trace=False):
    from concourse.bass_utils import run_bass_kernel_spmd

    x = np.asarray(x, dtype=np.float32)
    pos_bias = np.asarray(pos_bias, dtype=np.float32)
    W_qkv = np.asarray(W_qkv, dtype=np.float32)
    W_out = np.asarray(W_out, dtype=np.float32)

    if "nc" not in _CACHE:
        _CACHE["nc"] = _build()
    nc = _CACHE["nc"]

    in_maps = _host_inputs(x, pos_bias, W_qkv, W_out)
    try:
        res = run_bass_kernel_spmd(
            nc, in_maps, core_ids=list(range(HEADS)), trace=_trace
        )
    except ModuleNotFoundError:
        res = run_bass_kernel_spmd(
            nc, in_maps, core_ids=list(range(HEADS)), trace=False
        )
    out = np.zeros((B, N, DIM), dtype=np.float32)
    for rmap in res.results:
        out += np.asarray(rmap["out"], dtype=np.float32)
    if _trace:
        return out, res
    return out


if __name__ == "__main__":
    rng = np.random.default_rng(0)
    x = rng.standard_normal((B, N, DIM), dtype=np.float32)
    pb = rng.standard_normal((HEADS, N, N), dtype=np.float32)
    wq = rng.standard_normal((DIM, 3 * DIM), dtype=np.float32) * DIM**-0.5
    wo = rng.standard_normal((DIM, DIM), dtype=np.float32) * DIM**-0.5
    o = kernel(x, pb, wq, wo)
    print("kernel ran, out std:", o.std())


# revision 42
# speedup vs baseline: 1.1680x; 1.0082x over previous
"""Attention kernel for trn2: B=4, N=2048, DIM=512, HEADS=8, DIM_HEAD=64.

Sharding: head-parallel across 8 cores (core h computes head h for all 4
batches). Each core returns its head's contribution through W_out as a
bf16 partial [4, 2048, 512]; the host sums the 8 partials in fp32.

Per-core pipeline (bf16 data path, fp32 PSUM accumulation):
  phase 1: qkT = (W_qk)^T x^T computed directly in [d, n] layout; rotary
           applied via a +-1 permutation matmul (pair swap) plus 3 DVE
           ops against precomputed cos/sin tables; v in natural [n, d]
           layout with a ones column for row sums.
  phase 2: every matmul is contract-64 and runs 2x via PE row tiling
           (two concurrent 64x128 sub-array tiles at partitions 0:64 /
           64:128):
             S^T[k,q]: even key-chunks on tile (0,0) (k from kTlow,
             q from qkT rows 0:64), odd chunks on tile (64,0) (k from
             qkT rows 64:128, q from a duplicated copy at rows 64:128).
             exp on ACT (1024 wide), attn = et * exp(bias) on DVE (bf16,
             exp(bias) precomputed on host).
             PV: each 128-key chunk split into two 64-key halves run
             concurrently on both tiles into separate accumulators
             outT_A/outT_B (summed in the epilogue).
             W_out: two 128-row chunks run concurrently (ho duplicated
             to partitions 64:128 by SBUF-to-SBUF DMA).
"""

import itertools

import numpy as np

B, N, DIM = 4, 2048, 512
HEADS, DH = 8, 64
P = 128
DC = DIM // P          # 4 input-dim chunks
KC = N // P            # 16 key chunks
QT = 512               # query tile in phase 2
NQT = N // QT          # 4
NPAIR = KC // 2        # 8 key-chunk pairs

_CACHE = {}


def _build():
    import concourse.mybir as mybir
    import concourse.tile as tile
    from concourse import bacc

    F32 = mybir.dt.float32
    F32R = mybir.dt.float32r
    BF16 = mybir.dt.bfloat16

    nc = bacc.Bacc(None, target_bir_lowering=False)

    xT4_d = nc.dram_tensor("xT4", [B, P, DC, N], BF16, kind="ExternalInput")
    wqk_d = nc.dram_tensor("wqk", [P, DC, P], BF16, kind="ExternalInput")
    wv_d = nc.dram_tensor("wv", [P, DC, DH], BF16, kind="ExternalInput")
    st_d = nc.dram_tensor("st", [P, P], BF16, kind="ExternalInput")
    cosT_d = nc.dram_tensor("cosT", [P, N], BF16, kind="ExternalInput")
    sinT_d = nc.dram_tensor("sinT", [P, N], BF16, kind="ExternalInput")
    expb_d = nc.dram_tensor("expb", [KC, P, N], BF16, kind="ExternalInput")
    woutw_d = nc.dram_tensor("woutw", [DH, DIM], BF16, kind="ExternalInput")
    ones64_d = nc.dram_tensor("ones64", [DH, P], F32R, kind="ExternalInput")
    zeros64_d = nc.dram_tensor("zeros64", [DH, QT], F32R, kind="ExternalInput")
    out_d = nc.dram_tensor("out", [B, N, DIM], BF16, kind="ExternalOutput")

    with tile.TileContext(nc) as tc:
        with tc.tile_pool(name="const", bufs=1) as cp:
            wqk_t = cp.tile([P, DC, P], BF16, tag="wqk")
            nc.sync.dma_start(wqk_t[:], wqk_d[:, :, :])
            wv_t = cp.tile([P, DC, DH], BF16, tag="wv")
            nc.sync.dma_start(wv_t[:], wv_d[:, :, :])
            st_t = cp.tile([P, P], BF16, tag="st")
            nc.sync.dma_start(st_t[:], st_d[:, :])
            cosT_t = cp.tile([P, N], BF16, tag="cosT")
            sinT_t = cp.tile([P, N], BF16, tag="sinT")
            woutw2_t = cp.tile([P, DIM], BF16, tag="woutw2")
            ones64_t = cp.tile([DH, P], F32R, tag="ones64")
            rs64_t = [
                cp.tile([DH, QT], F32R, tag=f"rs64_{i}", name=f"rs64_{i}")
                for i in range(2)
            ]

            def load_late_consts():
                # deferred behind the startup-critical x(b=0) load
                nc.sync.dma_start(cosT_t[:], cosT_d[:, :])
                nc.sync.dma_start(sinT_t[:], sinT_d[:, :])
                # W_out rhs duplicated into both partition halves for the
                # row-tiled wout pair
                nc.sync.dma_start(woutw2_t[0:DH, :], woutw_d[:, :])
                nc.sync.dma_start(woutw2_t[DH:P, :], woutw_d[:, :])
                nc.sync.dma_start(ones64_t[:], ones64_d[:, :])
                # row-sum staging: row 0 written per (b, jq); rows 1:64
                # stay 0
                for t in rs64_t:
                    nc.sync.dma_start(t[:], zeros64_d[:, :])
            expb_t = [
                cp.tile([P, 2, N], BF16, tag=f"expb{t}", name=f"expb{t}")
                for t in range(NPAIR)
            ]

            def load_expb(ts):
                # staggered through phase 1 so the 8.4MB doesn't contend
                # with the startup-critical x/weight loads
                for t in ts:
                    nc.scalar.dma_start(
                        expb_t[t][:],
                        expb_d[2 * t : 2 * t + 2, :, :].rearrange("k p n -> p k n"),
                    )

            qkT_b = [
                cp.tile([P, N], BF16, tag=f"qkT{b}", name=f"qkT{b}") for b in range(B)
            ]
            kTlow_b = [
                cp.tile([DH, N], BF16, tag=f"kTlow{b}", name=f"kTlow{b}")
                for b in range(B)
            ]
            qhi_b = [
                cp.tile([P, N], BF16, tag=f"qhi{b}", name=f"qhi{b}")
                for b in range(B)
            ]
            v_b = [
                cp.tile([P, KC, DH + 2], BF16, tag=f"v{b}", name=f"v{b}")
                for b in range(B)
            ]

            # ---- phase 1: qkv projection (transposed) + rotary ----
            with (
                tc.tile_pool(name="p1", bufs=2) as p1,
                tc.tile_pool(name="pp1", bufs=2, space="PSUM") as pp1,
            ):
                def load_xt(b):
                    xt = p1.tile([P, DC, N], BF16, tag="xt", name=f"xt{b}")
                    for mc in range(N // QT):
                        ms = slice(mc * QT, (mc + 1) * QT)
                        nc.sync.dma_start(xt[:, :, ms], xT4_d[b, :, :, ms])
                    return xt

                xt_q = [load_xt(0)]
                load_late_consts()
                xt_q.append(load_xt(1))
                for b in range(B):
                    xt = xt_q[b]
                    for mc in range(N // QT):
                        ms = slice(mc * QT, (mc + 1) * QT)
                        qk_ps = pp1.tile([P, QT], F32, tag="qkps")
                        for dc in range(DC):
                            nc.tensor.matmul(
                                qk_ps[:],
                                lhsT=wqk_t[:, dc, :],
                                rhs=xt[:, dc, ms],
                                start=(dc == 0),
                                stop=(dc == DC - 1),
                            )
                        qk_sb = p1.tile([P, QT], BF16, tag="qksb")
                        nc.scalar.copy(qk_sb[:], qk_ps[:])
                        rot_ps = pp1.tile([P, QT], F32, tag="rotps")
                        nc.tensor.matmul(
                            rot_ps[:], lhsT=st_t[:], rhs=qk_sb[:],
                            start=True, stop=True,
                        )
                        # v for the four 128-row chunks of this column block;
                        # interleaved here so their weight loads hide under
                        # the long qk/rot matmuls
                        v_ps = pp1.tile([P, 4, DH], F32, tag="vps")
                        for i in range(4):
                            ch = mc * 4 + i
                            for dc in range(DC):
                                nc.tensor.matmul(
                                    v_ps[:, i, :],
                                    lhsT=xt[:, dc, ch * P : (ch + 1) * P],
                                    rhs=wv_t[:, dc, :],
                                    start=(dc == 0),
                                    stop=(dc == DC - 1),
                                )
                        a_t = p1.tile([P, QT], BF16, tag="a")
                        nc.vector.tensor_mul(a_t[:], qk_sb[:], cosT_t[:, ms])
                        rb_t = p1.tile([P, QT], BF16, tag="rb")
                        nc.vector.tensor_mul(rb_t[:], rot_ps[:], sinT_t[:, ms])
                        nc.vector.tensor_add(qkT_b[b][:, ms], a_t[:], rb_t[:])
                        # matmul operands must share a base partition: stage
                        # k at partitions 0:64 and a q copy at 64:128
                        nc.sync.dma_start(kTlow_b[b][:, ms], qkT_b[b][DH:P, ms])
                        nc.sync.dma_start(
                            qhi_b[b][DH:P, ms], qkT_b[b][0:DH, ms]
                        )
                        nc.scalar.copy(
                            v_b[b][:, mc * 4 : (mc + 1) * 4, 0:DH], v_ps[:]
                        )
                    nc.vector.memset(v_b[b][:, :, DH : DH + 1], 1.0)
                    load_expb(range(2 * b, 2 * b + 2))
                    if b + 2 < B:
                        xt_q.append(load_xt(b + 2))

            # ---- phase 2: attention (all matmuls 64x128 row-tiled) ----
            with (
                tc.tile_pool(name="p2", bufs=3) as p2,
                tc.tile_pool(name="ps2", bufs=2, space="PSUM") as ps2,
                tc.tile_pool(name="psa", bufs=1, space="PSUM") as psa,
                tc.tile_pool(name="psb", bufs=1, space="PSUM") as psb,
                tc.tile_pool(name="psw", bufs=2, space="PSUM") as psw,
            ):

                def epilogue_tail(b, jq, rs, outA, outB):
                    """Epilogue steps, one dependency hop per chunk so no
                    engine queue head-of-line blocks ahead of the next
                    block's exp/mult work. Both accumulators are drained
                    concurrently (ACT || DVE) so the next block's first PV
                    pair is released as fast as possible. The leading yield
                    delays the drains one step so the ACT copy lands after
                    the final PV pair has executed."""
                    yield
                    outA_sb = p2.tile(
                        [DH + 1, QT], F32, tag="outAsb", name="outA_sb", bufs=2
                    )
                    nc.scalar.copy(outA_sb[:], outA[:])
                    outB_sb = p2.tile(
                        [DH + 1, QT], F32, tag="outBsb", name="outB_sb", bufs=2
                    )
                    nc.vector.tensor_copy(outB_sb[:], outB[:])
                    yield
                    with nc.allow_low_precision(reason="f32r row-sum staging"):
                        nc.vector.tensor_add(
                            rs[0:1, :],
                            outA_sb[DH : DH + 1, :],
                            outB_sb[DH : DH + 1, :],
                        )
                    hsum = p2.tile([DH, QT], F32, tag="hsum", name="hsum", bufs=2)
                    nc.vector.tensor_add(
                        hsum[:], outA_sb[0:DH, :], outB_sb[0:DH, :]
                    )
                    yield
                    bc_ps = psw.tile([P, QT], F32, tag="wo", name="bc_ps")
                    nc.tensor.matmul(
                        bc_ps[:], lhsT=ones64_t[:], rhs=rs[:],
                        start=True, stop=True,
                    )
                    yield
                    bcr_sb = p2.tile([DH, QT], F32, tag="bcr", name="bcr_sb", bufs=2)
                    nc.vector.reciprocal(bcr_sb[:], bc_ps[0:DH, :])
                    yield
                    ho_t = p2.tile([P, QT], BF16, tag="ho", name="ho_t", bufs=2)
                    nc.vector.tensor_mul(ho_t[0:DH, :], hsum[:], bcr_sb[:])
                    yield
                    nc.sync.dma_start(ho_t[DH:P, :], ho_t[0:DH, :])
                    yield
                    for sqp in range(2):
                        wo_lo = psw.tile([P, DIM], F32, tag="wo", name="wo_lo")
                        nc.tensor.matmul(
                            wo_lo[:],
                            lhsT=ho_t[0:DH, (2 * sqp) * P : (2 * sqp + 1) * P],
                            rhs=woutw2_t[0:DH, :],
                            start=True,
                            stop=True,
                        )
                        wo_hi = psw.tile([P, DIM], F32, tag="wo", name="wo_hi")
                        nc.tensor.matmul(
                            wo_hi[:],
                            lhsT=ho_t[DH:P, (2 * sqp + 1) * P : (2 * sqp + 2) * P],
                            rhs=woutw2_t[DH:P, :],
                            start=True,
                            stop=True,
                        )
                        yield
                        for half, wo_ps in enumerate((wo_lo, wo_hi)):
                            ob = p2.tile([P, DIM], BF16, tag="ob", name="ob")
                            if half == 0:
                                nc.scalar.copy(ob[:], wo_ps[:])
                            else:
                                nc.vector.tensor_copy(ob[:], wo_ps[:])
                            row0 = jq * QT + (2 * sqp + half) * P
                            nc.sync.dma_start(out_d[b, row0 : row0 + P, :], ob[:])
                        yield

                # one continuous software-pipelined stream over all
                # (b, jq, pair) steps: PV lags S/exp/mult by LAG pairs even
                # across (b, jq) boundaries, so the S->exp->mult->PV chain
                # latency (~3us) amortizes instead of stalling each pair
                LAG = 4
                pending = None
                inflight = []
                outA = outB = None
                total = B * NQT * NPAIR
                for gi in range(total + LAG):
                    if gi < total:
                        bji, pp = divmod(gi, NPAIR)
                        b, jq = divmod(bji, NQT)
                        qs = slice(jq * QT, (jq + 1) * QT)
                        if pp == 0:
                            outA = psa.tile(
                                [DH + 1, QT], F32, tag="outA", name="outA"
                            )
                            outB = psb.tile(
                                [DH + 1, QT], F32, tag="outB", name="outB"
                            )
                        s_ps = ps2.tile([P, 2, QT], F32, tag="s", name="s_ps")
                        kce, kco = 2 * pp, 2 * pp + 1
                        nc.tensor.matmul(
                            s_ps[:, 0, :],
                            lhsT=kTlow_b[b][:, kce * P : (kce + 1) * P],
                            rhs=qkT_b[b][0:DH, qs],
                            start=True,
                            stop=True,
                        )
                        nc.tensor.matmul(
                            s_ps[:, 1, :],
                            lhsT=qkT_b[b][DH:P, kco * P : (kco + 1) * P],
                            rhs=qhi_b[b][DH:P, qs],
                            start=True,
                            stop=True,
                        )
                        et = p2.tile([P, 2, QT], BF16, tag="et", name="et", bufs=4)
                        nc.scalar.activation(
                            et[:], s_ps[:], mybir.ActivationFunctionType.Exp
                        )
                        if pending is not None:
                            next(pending, None)
                        at = p2.tile(
                            [P, 2, QT], BF16, tag="at", name="at", bufs=LAG + 2
                        )
                        nc.vector.tensor_mul(at[:], et[:], expb_t[pp][:, :, qs])
                        inflight.append((at, pp, b, jq, bji, outA, outB))
                    if len(inflight) > (LAG if gi < total else 0):
                        pat, ppp, pb, pjq, pbji, poutA, poutB = inflight.pop(0)
                        for i in range(2):
                            kc = 2 * ppp + i
                            nc.tensor.matmul(
                                poutA[:],
                                lhsT=v_b[pb][0:DH, kc, 0 : DH + 1],
                                rhs=pat[0:DH, i, :],
                                start=(kc == 0),
                                stop=(kc == KC - 1),
                            )
                            nc.tensor.matmul(
                                poutB[:],
                                lhsT=v_b[pb][DH:P, kc, 0 : DH + 1],
                                rhs=pat[DH:P, i, :],
                                start=(kc == 0),
                                stop=(kc == KC - 1),
                            )
                        if ppp == NPAIR - 1:
                            if pending is not None:
                                for _ in pending:
                                    pass
                            pending = epilogue_tail(
                                pb, pjq, rs64_t[pbji % 2], poutA, poutB
                            )
                    if gi >= total and pending is not None:
                        next(pending, None)
                for _ in pending:
                    pass

    nc.compile()
    return nc


def _host_inputs(x, pos_bias, W_qkv, W_out):
    """Build the per-core input maps (pure data marshalling)."""
    import ml_dtypes

    bf16 = ml_dtypes.bfloat16

    xT = x.transpose(0, 2, 1)                                # [B, DIM, N]
    xT4 = np.ascontiguousarray(
        xT.reshape(B, DC, P, N).transpose(0, 2, 1, 3)
    ).astype(bf16)                                           # [B, P, DC, N]

    # rotary tables in [d, n] layout, deinterleaved (evens then odds);
    # rows 0:32 q-even, 32:64 q-odd, 64:96 k-even, 96:128 k-odd.
    h = DH // 2
    inv_freq = 1.0 / (10000.0 ** (np.arange(0, DH, 2, dtype=np.float64) / DH))
    nn = np.arange(N, dtype=np.float64)
    cos32 = np.cos(inv_freq[:, None] * nn[None, :])          # [32, N]
    sin32 = np.sin(inv_freq[:, None] * nn[None, :])
    cosT = np.tile(cos32, (4, 1)).astype(bf16)               # [128, N]
    sinT = np.tile(sin32, (4, 1)).astype(bf16)

    # pair-swap permutation: rot[e_j] = -in[o_j], rot[o_j] = +in[e_j]
    # within each of the q (0:64) and k (64:128) blocks.
    S = np.zeros((P, P), dtype=np.float32)
    for base in (0, DH):
        for j in range(h):
            S[base + j, base + h + j] = -1.0
            S[base + h + j, base + j] = +1.0
    S_T = np.ascontiguousarray(S.T).astype(bf16)

    # broadcast matrix for the row-sum matmul: out rows 0:64 pick up
    # rs row 0 (rows 1:64 of rs are kept zero)
    ones64 = np.zeros((DH, P), dtype=np.float32)
    ones64[:, 0:DH] = 1.0

    scale = np.float32(DH**-0.5)
    deint = np.concatenate([np.arange(0, DH, 2), np.arange(1, DH, 2)])
    in_maps = []
    for hh in range(HEADS):
        Wq = W_qkv[:, hh * DH : (hh + 1) * DH] * scale
        Wk = W_qkv[:, DIM + hh * DH : DIM + (hh + 1) * DH]
        Wv = W_qkv[:, 2 * DIM + hh * DH : 2 * DIM + (hh + 1) * DH]
        Wqk = np.concatenate([Wq[:, deint], Wk[:, deint]], axis=1)  # [512, 128]
        wqk = np.ascontiguousarray(
            Wqk.reshape(DC, P, P).transpose(1, 0, 2)
        ).astype(bf16)                                       # [P, DC, P]
        wv = np.ascontiguousarray(
            Wv.reshape(DC, P, DH).transpose(1, 0, 2)
        ).astype(bf16)                                       # [P, DC, DH]
        expb = np.exp(pos_bias[hh].T).reshape(KC, P, N).astype(bf16)
        woutw = W_out[hh * DH : (hh + 1) * DH, :].astype(bf16)
        in_maps.append(
            {
                "xT4": xT4,
                "wqk": wqk,
                "wv": wv,
                "st": S_T,
                "cosT": cosT,
                "sinT": sinT,
                "expb": expb,
                "woutw": woutw,
                "ones64": ones64,
                "zeros64": np.zeros((DH, QT), dtype=np.float32),
            }
        )
    return in_maps


def _sim(x, pos_bias, W_qkv, W_out):
    """Numpy mirror of the device algorithm (for marshalling validation)."""
    import ml_dtypes

    bf16 = ml_dtypes.bfloat16
    in_maps = _host_inputs(x, pos_bias, W_qkv, W_out)
    out = np.zeros((B, N, DIM), dtype=np.float32)
    for hh in range(HEADS):
        m = in_maps[hh]
        wqk = m["wqk"].astype(np.float32).transpose(1, 0, 2).reshape(DIM, P)
        wv = m["wv"].astype(np.float32).transpose(1, 0, 2).reshape(DIM, DH)
        cosT = m["cosT"].astype(np.float32)
        sinT = m["sinT"].astype(np.float32)
        S_T = m["st"].astype(np.float32)
        for b in range(B):
            xb = m["xT4"][b].astype(np.float32).transpose(1, 0, 2).reshape(DIM, N)
            qkT = (wqk.T @ xb).astype(bf16).astype(np.float32)      # [128, N]
            rot = S_T.T @ qkT
            qkT = (
                (qkT * cosT).astype(bf16).astype(np.float32)
                + (rot * sinT).astype(bf16).astype(np.float32)
            ).astype(bf16).astype(np.float32)
            v = (xb.T @ wv).astype(bf16).astype(np.float32)         # [N, DH]
            qT, kT = qkT[0:DH], qkT[DH:P]
            s = kT.T @ qT                                           # [k, q]
            et = np.exp(s).astype(bf16).astype(np.float32)
            expb = (
                m["expb"].astype(np.float32).reshape(N, N)
            )                                                       # [k, q]
            at = (et * expb).astype(bf16).astype(np.float32)
            outT = np.concatenate([v, np.ones((N, 1), np.float32)], 1).T @ at
            ho = (
                (outT[0:DH] / outT[DH : DH + 1]).astype(bf16).astype(np.float32)
            )
            wo = ho.T @ m["woutw"].astype(np.float32)               # [N, DIM]
            out[b] += wo.astype(bf16).astype(np.float32)
    return out


def kernel(x, pos_bias, W_qkv, W_out, _trace=False):
    from concourse.bass_utils import run_bass_kernel_spmd

    x = np.asarray(x, dtype=np.float32)
    pos_bias = np.asarray(pos_bias, dtype=np.float32)
    W_qkv = np.asarray(W_qkv, dtype=np.float32)
    W_out = np.asarray(W_out, dtype=np.float32)

    if "nc" not in _CACHE:
        _CACHE["nc"] = _build()
    nc = _CACHE["nc"]

    in_maps = _host_inputs(x, pos_bias, W_qkv, W_out)
    try:
        res = run_bass_kernel_spmd(
            nc, in_maps, core_ids=list(range(HEADS)), trace=_trace
        )
    except ModuleNotFoundError:
        res = run_bass_kernel_spmd(
            nc, in_maps, core_ids=list(range(HEADS)), trace=False
        )
    out = np.zeros((B, N, DIM), dtype=np.float32)
    for rmap in res.results:
        out += np.asarray(rmap["out"], dtype=np.float32)
    if _trace:
        return out, res
    return out


if __name__ == "__main__":
    rng = np.random.default_rng(0)
    x = rng.standard_normal((B, N, DIM), dtype=np.float32)
    pb = rng.standard_normal((HEADS, N, N), dtype=np.float32)
    wq = rng.standard_normal((DIM, 3 * DIM), dtype=np.float32) * DIM**-0.5
    wo = rng.standard_normal((DIM, DIM), dtype=np.float32) * DIM**-0.5
    o = kernel(x, pb, wq, wo)
    print("kernel ran, out std:", o.std())
